# revision 34
# baseline (speedup 1.0000x reference)
"""CLIP attention (B=2, S=2048, H=768, 12 heads) on 8 trn2 NeuronCores.

Sharding: data-parallel over batch (2) x tensor-parallel over head groups
(4 groups of 3 heads).  Each core computes, for its (batch, head-group):
    q = x @ Wq_g * 1/sqrt(64) (+ bq_g scaled)      [2048, 192]
    k = x @ Wk_g                                    [2048, 192]
    v = x @ Wv_g                                    [2048, 192]
    per head: P' = exp(q k^T)   (no max subtraction; logits are O(1))
              O'^T, s via ones-augmented V:  o_ps = [V_h | 1]^T @ P'^T
    y_partial = sum_h (O'_h / s_h) @ Wo_h           [2048, 768]
Host sums the 4 head-group partials per batch and adds the exactly-folded
bias terms (bk drops out of softmax; bv/bo fold to a constant row).

Matmul convention: nc.tensor.matmul(out, lhsT, rhs) => out = lhsT.T @ rhs,
contraction over the partition dim of both operands.

Layout/scheduling:
  - x loaded contiguously in 256-token pair-kicks (split over both HWDGE
    queues), transposed on the PE array via is_transpose matmuls.
  - attention pass (h=0, qh=0) starts as soon as the first 512-token
    projection pieces land; the remaining transposes, q/k pieces (with
    per-512-column q^T/k^T duplication DMAs), V pieces, and head-2's
    packed q|k pieces drip into the pass's PE slack.
  - head-2's q and k projections packed into one 128-wide piece
    (w2 = [Wq2|Wk2]); k2 lands on partitions 64:127 so all PSUM->SBUF
    copies stay partition-aligned.
  - q^T/k^T stored twice (partitions 0-63 / 64-127) so two K=64 logits
    matmuls run concurrently in disjoint PE row groups.
  - PSUM: 2x lt (4 banks) + o_ps (2) + flex (2); the drip pool is
    1-bank tiles during head-0 passes, 2-bank Y tiles afterwards; the
    tail (head-2 second-half Y) gets its own 3x2-bank pool.
  - denominators bounced to [128, 8] per (head, half) via DRAM on the
    gpsimd SWDGE queue, then reciprocal -> per-partition Y scales.
  - output written bf16, first half on the sync queue, second half on
    the scalar queue (idle by then); host accumulates partials in f32.
"""

import sys

if "/opt/trn_rl_repo" not in sys.path:
    sys.path.insert(0, "/opt/trn_rl_repo")

from collections import deque

import numpy as np
import ml_dtypes

import concourse.bacc as bacc
import concourse.tile as tile
from concourse import mybir
from concourse.bass_utils import run_bass_kernel_spmd

BF16 = mybir.dt.bfloat16
F32 = mybir.dt.float32

S = 2048          # sequence length
C = 768           # hidden
NH = 12           # total heads
HD = 64           # head dim
NCORES = 8
GROUPS = 4        # head groups (tensor parallel)
HPG = NH // GROUPS          # heads per group = 3
GF = HPG * HD               # group feature width = 192
NCC = C // 128              # contraction chunks = 6
NQB = S // 128              # token blocks = 16
NKB = S // 128              # key blocks = 16


def build_program():
    nc = bacc.Bacc("TRN2", target_bir_lowering=False, debug=False)

    x = nc.dram_tensor("x", (S, C), BF16, kind="ExternalInput").ap()
    wq = nc.dram_tensor("wq", (C, GF), BF16, kind="ExternalInput").ap()
    wk = nc.dram_tensor("wk", (C, GF), BF16, kind="ExternalInput").ap()
    wv = nc.dram_tensor("wv", (C, GF), BF16, kind="ExternalInput").ap()
    wo = nc.dram_tensor("wo", (GF, C), BF16, kind="ExternalInput").ap()
    bq = nc.dram_tensor("bq", (1, GF), BF16, kind="ExternalInput").ap()
    ident = nc.dram_tensor("ident", (128, 128), BF16, kind="ExternalInput").ap()
    out = nc.dram_tensor("out", (S, C), BF16, kind="ExternalOutput").ap()

    wq_r = wq.rearrange("(n p) m -> p n m", p=128)
    wk_r = wk.rearrange("(n p) m -> p n m", p=128)
    wv_r = wv.rearrange("(n p) m -> p n m", p=128)
    x_r = x.rearrange("(s p) c -> p s c", p=128)

    with tile.TileContext(nc) as tc:
        with tc.tile_pool(name="consts", bufs=1) as consts, \
             tc.tile_pool(name="dram", bufs=1, space="DRAM") as dram_pool:
            # three parallel load queues:
            #   sync:   ident, x pairs (0,1) (2,3) (4,5)
            #   scalar: wq, wk, wv, x pairs (6,7) (8,9) (10,11), w2
            #   gpsimd: x pairs (12,13) (14,15), wo, biases
            id_sb = consts.tile([128, 128], BF16)
            nc.sync.dma_start(out=id_sb[:], in_=ident[:])

            wq_sb = consts.tile([128, NCC, 128], BF16)
            wk_sb = consts.tile([128, NCC, 128], BF16)
            w2_sb = consts.tile([128, NCC, 128], BF16)
            wv_sb = consts.tile([128, NCC, GF], BF16)
            nc.scalar.dma_start(out=wq_sb[:], in_=wq_r[:, :, 0:128])
            nc.scalar.dma_start(out=wk_sb[:], in_=wk_r[:, :, 0:128])
            nc.scalar.dma_start(out=wv_sb[:], in_=wv_r[:])

            x_sb = consts.tile([128, NQB, C], BF16)
            for tp in range(NQB // 2):
                eng = (nc.sync, nc.sync, nc.sync, nc.scalar,
                       nc.scalar, nc.scalar, nc.gpsimd, nc.gpsimd)[tp]
                eng.dma_start(
                    out=x_sb[:, 2 * tp : 2 * tp + 2, :],
                    in_=x_r[:, 2 * tp : 2 * tp + 2, :],
                )
            nc.scalar.dma_start(out=w2_sb[:, :, 0:64], in_=wq_r[:, :, 128:192])
            nc.scalar.dma_start(out=w2_sb[:, :, 64:128], in_=wk_r[:, :, 128:192])
            wo_t = [consts.tile([HD, C], BF16, name=f"wo{h}") for h in range(HPG)]
            for h in range(HPG):
                nc.gpsimd.dma_start(
                    out=wo_t[h][:], in_=wo[h * HD : (h + 1) * HD, :]
                )
            bq_sb = consts.tile([1, 128], BF16)
            nc.gpsimd.dma_start(out=bq_sb[:], in_=bq[:, 0:128])
            b2_sb = consts.tile([1, 128], BF16)
            nc.vector.memset(b2_sb[:], 0.0)
            nc.gpsimd.dma_start(out=b2_sb[:, 0:64], in_=bq[:, 128:192])

            ones_bf = consts.tile([1, 512], BF16)
            nc.vector.memset(ones_bf[:], 1.0)

            # x^T tiles per 512-token slab group: [128, c, 512]
            xT = [consts.tile([128, NCC, 512], BF16, name=f"xT{n}")
                  for n in range(4)]

            qT = [consts.tile([128, S], BF16, name=f"qT{h}") for h in range(HPG)]
            kT = [consts.tile([128, S], BF16, name=f"kT{h}") for h in range(HPG)]
            vS = [consts.tile([128, HPG, HD + 1], BF16, name=f"vS{t}")
                  for t in range(NKB)]
            # oT per (head, query-half); row 64 holds the softmax denom
            oT = [[consts.tile([HD + 1, S // 2], BF16, name=f"oT{h}_{qh}")
                   for qh in range(2)] for h in range(HPG)]
            sT = [consts.tile([128, NQB], F32, name=f"sT{h}") for h in range(HPG)]
            rT = [consts.tile([128, NQB], F32, name=f"rT{h}") for h in range(HPG)]
            ys = [consts.tile([128, C], F32, name=f"ys{t}") for t in range(NQB)]
            ys_bf = [consts.tile([128, C], BF16, name=f"ysb{t}")
                     for t in range(NQB)]
            zs_bf = {t: consts.tile([128, C], BF16, name=f"zsb{t}")
                     for t in range(NQB // 2 + 1, NQB, 2)}

            def transpose_slab(pool, t):
                tp = pool.tile([128, NCC, 128], BF16, tag="flex", name=f"xt{t}")
                for c in range(NCC):
                    nc.tensor.transpose(
                        tp[:, c, :],
                        x_sb[:, t, c * 128 : (c + 1) * 128],
                        id_sb[:],
                    )
                dst = xT[t // 4][:, :, (t % 4) * 128 : (t % 4 + 1) * 128]
                if t % 2 == 0:
                    nc.scalar.copy(dst, tp[:])
                else:
                    nc.vector.tensor_copy(dst, tp[:])

            def qk_proj_piece(pool, w_sb, with_bias, bias_sb, n, copies):
                """One [128, 512] projection piece; copies = list of
                (dst_tile, row0, src_row0) halves, each followed by its
                [64, 512] duplicate DMA into the other row group."""
                ps = pool.tile([128, 512], F32, tag="flex", name=f"qkp{n}")
                for c in range(NCC):
                    nc.tensor.matmul(
                        ps[:],
                        w_sb[:, c, :],
                        xT[n][:, c, :],
                        start=(c == 0),
                        stop=(c == NCC - 1 and not with_bias),
                    )
                if with_bias:
                    nc.tensor.matmul(
                        ps[:], bias_sb[:], ones_bf[:],
                        start=False, stop=True,
                    )
                n0 = n * 512
                for dst, dr0, sr0 in copies:
                    nc.vector.tensor_copy(
                        dst[dr0 : dr0 + 64, n0 : n0 + 512],
                        ps[sr0 : sr0 + 64, :],
                    )
                    nc.sync.dma_start(
                        out=dst[64 - dr0 : 128 - dr0, n0 : n0 + 512],
                        in_=dst[dr0 : dr0 + 64, n0 : n0 + 512],
                    )

            def v_piece(pool, t):
                vps = pool.tile([128, GF], F32, tag="flex", name=f"vp{t}")
                for c in range(NCC):
                    nc.tensor.matmul(
                        vps[:],
                        xT[t // 4][:, c, (t % 4) * 128 : (t % 4 + 1) * 128],
                        wv_sb[:, c, :],
                        start=(c == 0),
                        stop=(c == NCC - 1),
                    )
                nc.vector.tensor_copy(
                    vS[t][:, :, 0:HD],
                    vps[:].rearrange("p (h d) -> p h d", h=HPG),
                )
                nc.gpsimd.memset(vS[t][:, :, HD : HD + 1], 1.0)

            def y_mms(h, t, yp):
                tbs = slice((t % (NQB // 2)) * 128, (t % (NQB // 2) + 1) * 128)
                for n0, nw in ((0, 512), (512, 256)):
                    nc.tensor.matmul(
                        yp[:, n0 : n0 + nw],
                        oT[h][t // (NQB // 2)][0:HD, tbs],
                        wo_t[h][:, n0 : n0 + nw],
                        start=True,
                        stop=True,
                    )

            def y_finish(h, t, yp):
                if h == 0:
                    nc.vector.tensor_scalar_mul(
                        ys[t][:], yp[:], rT[0][:, t : t + 1]
                    )
                elif h == 1:
                    nc.vector.scalar_tensor_tensor(
                        out=ys[t][:],
                        in0=yp[:],
                        scalar=rT[1][:, t : t + 1],
                        in1=ys[t][:],
                        op0=mybir.AluOpType.mult,
                        op1=mybir.AluOpType.add,
                    )
                else:
                    if t in zs_bf:
                        # tail: ACT does the 1/s scale, Pool does the add,
                        # keeping the DVE free for the even-t steps
                        nc.scalar.mul(zs_bf[t][:], yp[:], rT[2][:, t : t + 1])
                        nc.gpsimd.tensor_add(ys_bf[t][:], zs_bf[t][:], ys[t][:])
                    else:
                        nc.vector.scalar_tensor_tensor(
                            out=ys_bf[t][:],
                            in0=yp[:],
                            scalar=rT[2][:, t : t + 1],
                            in1=ys[t][:],
                            op0=mybir.AluOpType.mult,
                            op1=mybir.AluOpType.add,
                        )
                    eng = nc.sync if (t < NQB // 2 or t % 2 == 0) else nc.scalar
                    eng.dma_start(
                        out=out[t * 128 : (t + 1) * 128, :],
                        in_=ys_bf[t][:],
                    )

            def attention_pass(h, qh, bg, ndrip=1):
                q0 = qh * 1024
                o_ps = opp.tile([HD + 1, 1024], F32, tag="o",
                                name=f"o_ps{h}_{qh}")
                for kb in range(NKB):
                    kbs = slice(kb * 128, (kb + 1) * 128)
                    lt = ltp.tile([128, 1024], F32, tag="lt")
                    # two concurrent K=64 matmuls in disjoint PE row groups
                    nc.tensor.matmul(
                        lt[:, 0:512],
                        kT[h][0:64, kbs],
                        qT[h][0:64, q0 : q0 + 512],
                        start=True,
                        stop=True,
                    )
                    nc.tensor.matmul(
                        lt[:, 512:1024],
                        kT[h][64:128, kbs],
                        qT[h][64:128, q0 + 512 : q0 + 1024],
                        start=True,
                        stop=True,
                    )
                    elt = asb.tile([128, 1024], BF16, tag="elt")
                    nc.scalar.activation(
                        elt[:], lt[:], mybir.ActivationFunctionType.Exp
                    )
                    for nn in range(2):
                        nc.tensor.matmul(
                            o_ps[:, nn * 512 : (nn + 1) * 512],
                            vS[kb][:, h, :],
                            elt[:, nn * 512 : (nn + 1) * 512],
                            start=(kb == 0),
                            stop=(kb == NKB - 1),
                        )
                    for _ in range(ndrip):
                        if bg:
                            bg.popleft()()
                nc.vector.tensor_copy(oT[h][qh][:], o_ps[0 : HD + 1, :])

                # denominators: PE-transpose oT blocks, take row 64 -> 1/s.
                # Returned as a closure so it can drip into the next pass
                # instead of stalling the PE at the pass boundary.
                def finalize(spool):
                    s_ps = spool.tile([128, 8, HD + 2], BF16, tag="flex",
                                      name=f"sps{h}_{qh}")
                    for b in range(8):
                        nc.tensor.transpose(
                            s_ps[:, b, 0 : HD + 1],
                            oT[h][qh][:, b * 128 : (b + 1) * 128],
                            id_sb[0 : HD + 1, 0 : HD + 1],
                        )
                    nc.vector.tensor_copy(
                        sT[h][:, qh * 8 : (qh + 1) * 8],
                        s_ps[:, :, HD],
                    )
                    nc.vector.reciprocal_approx_fast(
                        out=rT[h][:, qh * 8 : (qh + 1) * 8],
                        in_=sT[h][:, qh * 8 : (qh + 1) * 8],
                    )
                return finalize

            # ---- upfront: first-half transposes + n0/n1 projections ----
            with tc.tile_pool(name="upf", bufs=4, space="PSUM") as upf:
                qcopies = [(qT[0], 0, 0), (qT[1], 0, 64)]
                kcopies = [(kT[0], 0, 0), (kT[1], 0, 64)]
                for n in range(2):
                    for t in range(4 * n, 4 * n + 4):
                        transpose_slab(upf, t)
                    qk_proj_piece(upf, wq_sb, True, bq_sb, n, qcopies)
                    qk_proj_piece(upf, wk_sb, False, None, n, kcopies)
                    v_piece(upf, 2 * n)
                    v_piece(upf, 2 * n + 1)
                for t in range(4, 8):
                    v_piece(upf, t)

            with tc.tile_pool(name="lt_ps", bufs=2, space="PSUM") as ltp, \
                 tc.tile_pool(name="o_ps", bufs=1, space="PSUM") as opp, \
                 tc.tile_pool(name="att_sb", bufs=4) as asb:

                # ---- head-0 passes + small-tile drips (1-bank flex) ----
                with tc.tile_pool(name="flexA", bufs=2, space="PSUM") as flexA:
                    bgA = deque()

                    def A(f, *a):
                        bgA.append(lambda: f(flexA, *a))

                    # pass (0,0) pops 2/slot; V_t must be emitted strictly
                    # before its consumer slot kb=t, transposes before
                    # their pieces, q-n3 before pass (0,1) begins
                    A(transpose_slab, 8)
                    A(transpose_slab, 9)
                    A(transpose_slab, 10)
                    A(transpose_slab, 11)
                    A(qk_proj_piece, wq_sb, True, bq_sb, 2, qcopies)
                    A(qk_proj_piece, wk_sb, False, None, 2, kcopies)
                    A(v_piece, 8)
                    A(v_piece, 9)
                    A(transpose_slab, 12)
                    A(transpose_slab, 13)
                    A(transpose_slab, 14)
                    A(transpose_slab, 15)
                    A(qk_proj_piece, wq_sb, True, bq_sb, 3, qcopies)
                    A(qk_proj_piece, wk_sb, False, None, 3, kcopies)
                    A(v_piece, 10)
                    A(v_piece, 11)
                    A(v_piece, 12)
                    A(v_piece, 13)
                    A(v_piece, 14)
                    A(v_piece, 15)
                    fin00 = attention_pass(0, 0, bgA, ndrip=2)
                    # head-2 packed q|k pieces drip into pass (0,1):
                    # q2 -> qT[2] rows 0:64, k2 -> kT[2] rows 64:128
                    m2qk = [(qT[2], 0, 0), (kT[2], 64, 64)]
                    for n in range(4):
                        A(qk_proj_piece, w2_sb, True, b2_sb, n, m2qk)
                    bgA.appendleft(lambda: fin00(flexA))
                    fin01 = attention_pass(0, 1, bgA)
                    while bgA:
                        bgA.popleft()()

                # ---- head-1/2 passes + Y drips (2-bank flex) ----
                with tc.tile_pool(name="flexB", bufs=1, space="PSUM") as flexB:
                    bgB = deque()

                    def bg_y_step(h, t):
                        def run():
                            yp = flexB.tile([128, C], F32, tag="flex",
                                            name=f"yp{h}_{t}")
                            y_mms(h, t, yp)
                            y_finish(h, t, yp)
                        return run

                    for t in range(NQB // 2):
                        bgB.append(bg_y_step(0, t))
                    bgB.appendleft(lambda: fin01(flexB))
                    fin10 = attention_pass(1, 0, bgB)
                    for t in range(NQB // 2, NQB):
                        bgB.append(bg_y_step(0, t))
                    bgB.appendleft(lambda: fin10(flexB))
                    fin11 = attention_pass(1, 1, bgB)
                    for t in range(12):
                        bgB.append(bg_y_step(1, t))
                    bgB.appendleft(lambda: fin11(flexB))
                    fin20 = attention_pass(2, 0, bgB)
                    for t in range(12, NQB):
                        bgB.append(bg_y_step(1, t))
                    for t in range(NQB // 2):
                        bgB.append(bg_y_step(2, t))
                    bgB.appendleft(lambda: fin20(flexB))
                    fin21 = attention_pass(2, 1, bgB)
                    fin21(flexB)
                    while bgB:
                        bgB.popleft()()

            # ---- tail: head-2 Y, second query half (own 6-bank pool) ----
            with tc.tile_pool(name="tail", bufs=3, space="PSUM") as tailp:
                for t in range(NQB // 2, NQB):
                    yp = tailp.tile([128, C], F32, tag="tail", name=f"ypt{t}")
                    y_mms(2, t, yp)
                    y_finish(2, t, yp)

    nc.compile()
    return nc


_COMPILED_NC = None


def _get_nc():
    global _COMPILED_NC
    if _COMPILED_NC is None:
        _COMPILED_NC = build_program()
    return _COMPILED_NC


def make_in_maps(x, Wq, bq, Wk, bk, Wv, bv, Wo, bo):
    scale = 1.0 / np.sqrt(HD)
    bf = ml_dtypes.bfloat16
    x_bf = [np.ascontiguousarray(x[b]).astype(bf) for b in range(x.shape[0])]
    eye = np.eye(128, dtype=bf)
    in_maps = []
    for c in range(NCORES):
        b, g = divmod(c, GROUPS)
        cols = slice(g * GF, (g + 1) * GF)
        in_maps.append(
            {
                "x": x_bf[b],
                "wq": np.ascontiguousarray(Wq[:, cols] * scale).astype(bf),
                "wk": np.ascontiguousarray(Wk[:, cols]).astype(bf),
                "wv": np.ascontiguousarray(Wv[:, cols]).astype(bf),
                "wo": np.ascontiguousarray(Wo[cols, :]).astype(bf),
                "bq": np.ascontiguousarray(bq[cols] * scale).reshape(1, GF).astype(bf),
                "ident": eye,
            }
        )
    return in_maps


def gather_output(results, x, Wv, bv, Wo, bo):
    B = x.shape[0]
    out = np.zeros((B, S, C), dtype=np.float32)
    for c in range(NCORES):
        b, _ = divmod(c, GROUPS)
        out[b] += np.asarray(results[c]["out"], dtype=np.float32)
    # exact bias folds: bk cancels in softmax; v-bias -> bv @ Wo; + bo
    out += (np.asarray(bv, np.float32) @ np.asarray(Wo, np.float32)
            + np.asarray(bo, np.float32))
    return out


def kernel(x, Wq, bq, Wk, bk, Wv, bv, Wo, bo):
    x = np.asarray(x)
    nc = _get_nc()
    in_maps = make_in_maps(x, Wq, bq, Wk, bk, Wv, bv, Wo, bo)
    res = run_bass_kernel_spmd(nc, in_maps, core_ids=list(range(NCORES)))
    return gather_output(res.results, x, Wv, bv, Wo, bo)
